# revision 64
# baseline (speedup 1.0000x reference)
"""Sparse-attention distance-mask kernel for Trainium2 (8 NeuronCores).

Reference computation (per batch b):
    pos      = multi-hot of 4 tree-position ids over 512 nodes   [seq, 512]
    dist     = s_i + s_j - 2 * pos @ pos.T          (L1 dist of binary vecs)
    attn     = max(dist_top, dist_left)
    out      = attn + padding_dist * max(pad_i, pad_j)

Kernel strategy (measured 48243ns baseline -> ~29100ns):
  - Data-parallel over batch: core c computes batch c (b == n_cores == 8).
  - Device computes E = pos@pos.T - a/2 per mask, where
    a = s_i + s_j + p*(pad_i + pad_j - pad_i*pad_j); then
    out = -2*min(E_top, E_left) (host applies the -2 and the mirror).
    Sharing ONE unscaled pos array between lhsT and rhs halves input DMA.
  - pos operands are [128, 4, 1024] fp8 k-tile-major; the 4 k-tiles are
    contracted as 2 DoubleRow passes (K=256/pass, measured at the same
    215ns as a K=128 pass) -> 2x fewer PE passes.
  - The -a/2 term is a rank-5 fp8 augmentation k-tile per mask, one full
    K=128 pass per block (K=32 aug measured ~50% slower -- FWL needs 128
    weights).  Rows (c1*c2 == p/2):
      lhs: [-s_i/2, 1, -c1*pad_i, -c1, c1*pad_i]
      rhs: [1, -s_j/2, c2, c2*pad_j, c2*pad_j]
    bf16 3-row fallback when p/2 has no fp8-exact factorization.  The
    top-aug tile ships fully pre-zeroed over the wire (no memset gates
    its DMA -- it's on the critical path); the left-aug ships only its 5
    live rows onto a gpsimd-memset tile.
  - Symmetry at 128-row granularity: 4608 of 8192 [128,128] blocklets
    computed (36/64 upper-triangle + diagonal); mirrored on host.
  - Epilogue: ACT copies top-PSUM -> SBUF fp32; DVE min(SBUF, left-PSUM)
    -> per-band bf16 tiles (2E is an integer with |2E| <= 108 -> bf16
    exact); ONE store per band on alternating Sync/ACT HWDGE queues
    (each dma_start costs ~650ns of descriptor-gen on its sequencer, so
    8 triggers beat 12-14), ending on the 128-wide mb7 band for a short
    MIN+store tail.
  - 7 warm-up matmuls on gpsimd-memset scratch release the PE HAM clock
    gate (K=4/8 -> 8/8) while the input DMA streams in; the left-mask
    GEMM of block k-3 interleaves after the top-mask GEMM of block k,
    spreading the DVE min chain over the whole PE span; the first three
    blocks' kt-pair-0 passes are batched up front so the PE starts on
    ptop chunk 0 without waiting for chunk 1.
  - PSUM: one shared full-bank tag, 6 bufs in flight + 1 warm-up bank.
"""

import os

import ml_dtypes
import numpy as np

B, SEQ, DEPTH = 8, 1024, 4
TN = 512          # TOTAL_NODE
N_CORES = 8
MB = SEQ // 128
# per 128-row band, the first computed column (cols below are strictly under
# the diagonal and mirrored on host at 128-granularity)
ROW_LO = {mb: mb * 128 for mb in range(MB)}
ROW_BLOCKS = {}
for mb in range(MB):
    lo = ROW_LO[mb]
    blocks = []
    if lo % 512:
        blocks.append((lo, 512 - lo % 512))
        lo += 512 - lo % 512
    while lo < SEQ:
        blocks.append((lo, 512))
        lo += 512
    ROW_BLOCKS[mb] = blocks

_NC_CACHE = {}
LAST_RESULTS = None


def _build_nc(fused):
    import concourse.mybir as mybir
    from concourse import bacc
    from concourse.tile import TileContext

    aug_dt = mybir.dt.float8e4 if fused else mybir.dt.bfloat16
    nc = bacc.Bacc()
    dram = {
        "pos_top": nc.dram_tensor(
            "pos_top", [2, 128, 2, 1024], mybir.dt.float8e4,
            kind="ExternalInput"),
        "pos_left": nc.dram_tensor(
            "pos_left", [128, 4096], mybir.dt.float8e4, kind="ExternalInput"),
        "augs_top": nc.dram_tensor(
            "augs_top", [128, 2048], aug_dt, kind="ExternalInput"),
        "augs_left": nc.dram_tensor(
            "augs_left", [5, 2048], aug_dt, kind="ExternalInput"),
    }
    out = nc.dram_tensor("out", [SEQ, SEQ], mybir.dt.bfloat16,
                         kind="ExternalOutput")

    DR = mybir.MatmulPerfMode.DoubleRow

    with TileContext(nc) as tc:
        with (
            tc.tile_pool(name="w", bufs=1) as wpool,
            tc.tile_pool(name="ps", bufs=2, space="PSUM") as ppool,
            tc.tile_pool(name="ep", bufs=1) as epool,
            tc.tile_pool(name="ob", bufs=6) as opool,
        ):
            pos = {
                "top": wpool.tile([128, 4, SEQ], mybir.dt.float8e4,
                                  tag="ptop", name="ptop"),
                "left": wpool.tile([128, 4, SEQ], mybir.dt.float8e4,
                                   tag="pleft", name="pleft"),
            }
            augT = wpool.tile([128, 2, SEQ], aug_dt, tag="augT", name="augT")
            augL = wpool.tile([128, 2, SEQ], aug_dt, tag="augL", name="augL")
            scratch = wpool.tile([128, 640], mybir.dt.float8e4,
                                 tag="scratch", name="scratch")

            # warm-up scratch zeroed on the otherwise-idle gpsimd engine
            nc.gpsimd.memset(scratch[:, :], 0.0)
            # left-aug zero rows built on the idle gpsimd engine; only its 5
            # live K-rows ship over the wire.  The top-aug (on the critical
            # path) ships fully pre-zeroed so no memset gates its DMA.
            nc.gpsimd.memset(augL[:, :, :], 0.0)

            # input DMA, all on the Sync HWDGE queue in priority order;
            # descriptors fan out across all 16 DMA engines
            for ck in range(2):
                nc.sync.dma_start(out=pos["top"][:, 2 * ck:2 * ck + 2, :],
                                  in_=dram["pos_top"][ck])
            nc.sync.dma_start(out=augT[:, :, :], in_=dram["augs_top"][:, :])
            nc.sync.dma_start(out=pos["left"][:, :, :],
                              in_=dram["pos_left"][:, :])
            nc.sync.dma_start(out=augL[0:5, :, :], in_=dram["augs_left"][:, :])

            # PE warm-up (results never read): releases the HAM clock gate
            # during the DMA fill
            ps_w = ppool.tile([128, 512], mybir.dt.float32, tag="pw",
                              name="ps_warm", bufs=1)
            for _ in range(8):
                nc.tensor.matmul(ps_w[:, :], lhsT=scratch[:, 0:128],
                                 rhs=scratch[:, 128:640],
                                 start=True, stop=True)

            # cp (fp32 top-mask) and obf (bf16 min) tiles: one per row band
            cps = {}
            obfs = {}
            for mb in range(MB):
                wid = SEQ - ROW_LO[mb]
                # bf16 cp is exact (2E integer, |2E| <= 108) and halves the
                # DVE min's SBUF operand bytes + the ACT copy cost
                cps[mb] = epool.tile([128, wid], mybir.dt.bfloat16,
                                     tag=f"cp{mb}", name=f"cp{mb}")
                obfs[mb] = epool.tile([128, wid], mybir.dt.bfloat16,
                                      tag=f"ob{mb}", name=f"ob{mb}")

            def cp_slice(mb, c0, w):
                off = c0 - ROW_LO[mb]
                return cps[mb][:, off:off + w]

            def ob_slice(mb, c0, w):
                off = c0 - ROW_LO[mb]
                return obfs[mb][:, off:off + w]

            def psum_for(w, nm):
                # one shared full-bank tag; narrow blocks slice it
                t = ppool.tile([128, 512], mybir.dt.float32, tag="pmm",
                               name=nm, bufs=6)
                return t[:, :w]

            def dr_pass(ps, key, mb, c0, w, t0):
                m0 = mb * 128
                nc.tensor.matmul(
                    ps[:, :],
                    lhsT=pos[key][:, t0:t0 + 2, m0:m0 + 128],
                    rhs=pos[key][:, t0:t0 + 2, c0:c0 + w],
                    start=(t0 == 0), stop=False, perf_mode=DR,
                )

            def dr_passes(ps, key, mb, c0, w):
                for t0 in (0, 2):
                    dr_pass(ps, key, mb, c0, w, t0)

            def aug_pass(ps, aug_l, aug_r, mb, c0, w):
                m0 = mb * 128
                at = augT if aug_l == 0 else augL
                nc.tensor.matmul(
                    ps[:, :],
                    lhsT=at[:, 0:1, m0:m0 + 128],
                    rhs=at[:, 1:2, c0:c0 + w],
                    start=False, stop=True, skip_group_check=True,
                )

            # big blocks first, 256-wide blocks last (short MIN+store tail)
            ordA = [(mb, c0, w) for mb in range(MB)
                    for (c0, w) in ROW_BLOCKS[mb]]
            NB = len(ordA)

            def gemm_a(k):
                mb, c0, w = ordA[k]
                ps = psum_for(w, f"pt{mb}_{c0}")
                dr_passes(ps, "top", mb, c0, w)
                aug_pass(ps, 0, 1, mb, c0, w)
                nc.scalar.copy(cp_slice(mb, c0, w), ps[:, :])

            def gemm_b(k):
                mb, c0, w = ordA[k]
                ps = psum_for(w, f"pl{mb}_{c0}")
                dr_passes(ps, "left", mb, c0, w)
                aug_pass(ps, 2, 3, mb, c0, w)
                nc.vector.tensor_tensor(
                    out=ob_slice(mb, c0, w), in0=cp_slice(mb, c0, w),
                    in1=ps[:, :], op=mybir.AluOpType.min,
                )
                # one store per BAND (the ~650ns descriptor-gen per
                # dma_start serializes on the sequencer; 8 triggers beat 12)
                if c0 + w == SEQ:
                    r0 = mb * 128
                    lo = ROW_LO[mb]
                    eng = nc.sync if mb % 2 == 1 else nc.scalar
                    eng.dma_start(out=out[r0:r0 + 128, lo:SEQ],
                                  in_=obfs[mb][:, :])

            # top/left interleaved at block granularity: left GEMMs lag by
            # LAG blocks so the left-pos/aug DMA has landed, and the DVE MIN
            # chain spreads over the whole PE span instead of the back half
            LAG = 3
            # first 3 blocks: batch their kt-pair-0 DR passes so the PE can
            # start on ptop chunk 0 without waiting for chunk 1 / aug
            HEAD = 3
            head_ps = []
            for k in range(HEAD):
                mb, c0, w = ordA[k]
                ps = psum_for(w, f"pt{mb}_{c0}")
                head_ps.append(ps)
                dr_pass(ps, "top", mb, c0, w, 0)
            for k in range(HEAD):
                mb, c0, w = ordA[k]
                ps = head_ps[k]
                dr_pass(ps, "top", mb, c0, w, 2)
                aug_pass(ps, 0, 1, mb, c0, w)
                nc.scalar.copy(cp_slice(mb, c0, w), ps[:, :])
            for k in range(HEAD, LAG):
                gemm_a(k)
            for k in range(LAG, NB):
                gemm_b(k - LAG)
                gemm_a(k)
            for k in range(NB - LAG, NB):
                gemm_b(k)
    nc.compile()
    return nc


def _fp8_exact(x):
    f = x.astype(ml_dtypes.float8_e4m3).astype(np.float32)
    return np.array_equal(f, x)


def _aug_factor(ph):
    """Find c1*c2 == ph with c1, c2 fp8(e4m3)-exact; None if impossible."""
    for k in range(-6, 8):
        for m in range(8):
            c2 = np.float32(2.0 ** k) * np.float32(1 + m / 8.0)
            if c2 == 0:
                continue
            c1 = np.float32(ph) / c2
            cand = np.array([c1, c2], dtype=np.float32)
            if c1 * c2 == np.float32(ph) and _fp8_exact(cand):
                return float(c1), float(c2)
    return None


def _host_prep(zipped_top, zipped_left, indicator, p):
    """Build fp8 pos operands + aug k-tiles; returns (ins, fused)."""
    fp8 = ml_dtypes.float8_e4m3
    pos = {}
    s = {}
    for key, zipped in (("top", zipped_top), ("left", zipped_left)):
        b, seq, depth = zipped.shape
        oh = np.zeros((b, seq, TN + 1), dtype=np.float32)
        np.put_along_axis(oh, np.asarray(zipped, dtype=np.int64), 1.0, axis=2)
        oh = oh[..., :TN]
        s[key] = oh.sum(axis=2)                              # [b, seq]
        # [b, p, kt, j] k-tile-major
        pos[key] = oh.transpose(0, 2, 1).reshape(b, 4, 128, seq).transpose(
            0, 2, 1, 3)
    pad = (np.asarray(indicator) == 0).astype(np.float32)    # [b, seq]
    b, seq = pad.shape

    ph = np.float32(p) / np.float32(2.0)
    # s/2 is fp8-exact (multiples of 0.5 up to 2); need ph = c1*c2 exact
    fac = _aug_factor(ph) if _fp8_exact(s["top"] / 2) else None
    fused = fac is not None

    # aug k-tiles [b, 5, set, seq]; rows 5..127 are zeroed on-chip
    aug = np.zeros((b, 5, 4, seq), dtype=np.float32)
    for mi, key in enumerate(("top", "left")):
        sl, sr = 2 * mi, 2 * mi + 1
        if fused:
            c1, c2 = fac
            aug[:, 0, sl] = -s[key] / 2
            aug[:, 1, sl] = 1.0
            aug[:, 2, sl] = -c1 * pad
            aug[:, 3, sl] = -c1
            aug[:, 4, sl] = c1 * pad
            aug[:, 0, sr] = 1.0
            aug[:, 1, sr] = -s[key] / 2
            aug[:, 2, sr] = c2
            aug[:, 3, sr] = c2 * pad
            aug[:, 4, sr] = c2 * pad
        else:
            g = s[key] / np.float32(2.0) + ph * pad
            aug[:, 0, sl] = -g
            aug[:, 1, sl] = 1.0
            aug[:, 2, sl] = ph * pad
            aug[:, 0, sr] = 1.0
            aug[:, 1, sr] = -g
            aug[:, 2, sr] = pad
    aug_dt = fp8 if fused else ml_dtypes.bfloat16
    # chunk-major: [b, 2, 128, 2048] with chunk = mask (sets 01 | 23)
    augs5 = np.ascontiguousarray(
        aug.reshape(b, 5, 2, 2 * seq).transpose(0, 2, 1, 3))  # [b,2,5,2048]
    augs_top = np.zeros((b, 128, 2 * seq), dtype=np.float32)
    augs_top[:, 0:5] = augs5[:, 0]

    ins = {
        "pos_top": np.ascontiguousarray(
            pos["top"].reshape(b, 128, 2, 2 * seq).transpose(0, 2, 1, 3)
        ).astype(fp8),
        "pos_left": np.ascontiguousarray(
            pos["left"].reshape(b, 128, 4 * seq)).astype(fp8),
        "augs_top": augs_top.astype(aug_dt),
        "augs_left": np.ascontiguousarray(augs5[:, 1]).astype(aug_dt),
    }
    return ins, fused


def kernel(zipped_top, zipped_left, indicator, padding_dist):
    global LAST_RESULTS
    from concourse.bass_utils import run_bass_kernel_spmd

    p = float(np.asarray(padding_dist))
    ins, fused = _host_prep(
        np.asarray(zipped_top), np.asarray(zipped_left), indicator, p)

    if fused not in _NC_CACHE:
        _NC_CACHE[fused] = _build_nc(fused)
    nc = _NC_CACHE[fused]

    in_maps = [{k: v[c] for k, v in ins.items()} for c in range(N_CORES)]
    res = run_bass_kernel_spmd(
        nc, in_maps, core_ids=list(range(N_CORES)),
        trace=os.environ.get("BASS_TRACE", "") == "1",
    )
    LAST_RESULTS = res
    full = np.stack([res.results[c]["out"] for c in range(N_CORES)]).astype(
        np.float32
    )
    full *= np.float32(-2.0)
    # mirror the skipped below-diagonal region of each band
    for mb in range(MB):
        lo = ROW_LO[mb]
        if lo:
            r = slice(mb * 128, (mb + 1) * 128)
            full[:, r, :lo] = full[:, :lo, r].transpose(0, 2, 1)
    return full


# revision 65
# speedup vs baseline: 1.0101x; 1.0101x over previous
"""Sparse-attention distance-mask kernel for Trainium2 (8 NeuronCores).

Reference computation (per batch b):
    pos      = multi-hot of 4 tree-position ids over 512 nodes   [seq, 512]
    dist     = s_i + s_j - 2 * pos @ pos.T          (L1 dist of binary vecs)
    attn     = max(dist_top, dist_left)
    out      = attn + padding_dist * max(pad_i, pad_j)

Kernel strategy (measured 48243ns baseline -> ~29100ns):
  - Data-parallel over batch: core c computes batch c (b == n_cores == 8).
  - Device computes E = pos@pos.T - a/2 per mask, where
    a = s_i + s_j + p*(pad_i + pad_j - pad_i*pad_j); then
    out = -2*min(E_top, E_left) (host applies the -2 and the mirror).
    Sharing ONE unscaled pos array between lhsT and rhs halves input DMA.
  - pos operands are [128, 4, 1024] fp8 k-tile-major; the 4 k-tiles are
    contracted as 2 DoubleRow passes (K=256/pass, measured at the same
    215ns as a K=128 pass) -> 2x fewer PE passes.
  - The -a/2 term is a rank-5 fp8 augmentation k-tile per mask, one full
    K=128 pass per block (K=32 aug measured ~50% slower -- FWL needs 128
    weights).  Rows (c1*c2 == p/2):
      lhs: [-s_i/2, 1, -c1*pad_i, -c1, c1*pad_i]
      rhs: [1, -s_j/2, c2, c2*pad_j, c2*pad_j]
    bf16 3-row fallback when p/2 has no fp8-exact factorization.  The
    top-aug tile ships fully pre-zeroed over the wire (no memset gates
    its DMA -- it's on the critical path); the left-aug ships only its 5
    live rows onto a gpsimd-memset tile.
  - Symmetry at 128-row granularity: 4608 of 8192 [128,128] blocklets
    computed (36/64 upper-triangle + diagonal); mirrored on host.
  - Epilogue: ACT copies top-PSUM -> SBUF fp32; DVE min(SBUF, left-PSUM)
    -> per-band bf16 tiles (2E is an integer with |2E| <= 108 -> bf16
    exact); ONE store per band on alternating Sync/ACT HWDGE queues
    (each dma_start costs ~650ns of descriptor-gen on its sequencer, so
    8 triggers beat 12-14), ending on the 128-wide mb7 band for a short
    MIN+store tail.
  - 7 warm-up matmuls on gpsimd-memset scratch release the PE HAM clock
    gate (K=4/8 -> 8/8) while the input DMA streams in; the left-mask
    GEMM of block k-3 interleaves after the top-mask GEMM of block k,
    spreading the DVE min chain over the whole PE span; the first three
    blocks' kt-pair-0 passes are batched up front so the PE starts on
    ptop chunk 0 without waiting for chunk 1.
  - PSUM: one shared full-bank tag, 6 bufs in flight + 1 warm-up bank.
"""

import os

import ml_dtypes
import numpy as np

B, SEQ, DEPTH = 8, 1024, 4
TN = 512          # TOTAL_NODE
N_CORES = 8
MB = SEQ // 128
# per 128-row band, the first computed column (cols below are strictly under
# the diagonal and mirrored on host at 128-granularity)
ROW_LO = {mb: mb * 128 for mb in range(MB)}
ROW_BLOCKS = {}
for mb in range(MB):
    lo = ROW_LO[mb]
    blocks = []
    if lo % 512:
        blocks.append((lo, 512 - lo % 512))
        lo += 512 - lo % 512
    while lo < SEQ:
        blocks.append((lo, 512))
        lo += 512
    ROW_BLOCKS[mb] = blocks

_NC_CACHE = {}
LAST_RESULTS = None


def _build_nc(fused):
    import concourse.mybir as mybir
    from concourse import bacc
    from concourse.tile import TileContext

    aug_dt = mybir.dt.float8e4 if fused else mybir.dt.bfloat16
    nc = bacc.Bacc()
    dram = {
        "pos_top": nc.dram_tensor(
            "pos_top", [2, 128, 2, 1024], mybir.dt.float8e4,
            kind="ExternalInput"),
        "pos_left": nc.dram_tensor(
            "pos_left", [128, 4096], mybir.dt.float8e4, kind="ExternalInput"),
        "augs_top": nc.dram_tensor(
            "augs_top", [128, 2048], aug_dt, kind="ExternalInput"),
        "augs_left": nc.dram_tensor(
            "augs_left", [5, 2048], aug_dt, kind="ExternalInput"),
    }
    out = nc.dram_tensor("out", [SEQ, SEQ], mybir.dt.bfloat16,
                         kind="ExternalOutput")

    DR = mybir.MatmulPerfMode.DoubleRow

    with TileContext(nc) as tc:
        with (
            tc.tile_pool(name="w", bufs=1) as wpool,
            tc.tile_pool(name="ps", bufs=2, space="PSUM") as ppool,
            tc.tile_pool(name="ep", bufs=1) as epool,
            tc.tile_pool(name="ob", bufs=6) as opool,
        ):
            pos = {
                "top": wpool.tile([128, 4, SEQ], mybir.dt.float8e4,
                                  tag="ptop", name="ptop"),
                "left": wpool.tile([128, 4, SEQ], mybir.dt.float8e4,
                                   tag="pleft", name="pleft"),
            }
            augT = wpool.tile([128, 2, SEQ], aug_dt, tag="augT", name="augT")
            augL = wpool.tile([128, 2, SEQ], aug_dt, tag="augL", name="augL")
            scratch = wpool.tile([128, 640], mybir.dt.float8e4,
                                 tag="scratch", name="scratch")

            # warm-up scratch zeroed on the otherwise-idle gpsimd engine
            nc.gpsimd.memset(scratch[:, :], 0.0)
            # left-aug zero rows built on the idle gpsimd engine; only its 5
            # live K-rows ship over the wire.  The top-aug (on the critical
            # path) ships fully pre-zeroed so no memset gates its DMA.
            nc.gpsimd.memset(augL[:, :, :], 0.0)

            # input DMA, all on the Sync HWDGE queue in priority order;
            # descriptors fan out across all 16 DMA engines
            for ck in range(2):
                nc.sync.dma_start(out=pos["top"][:, 2 * ck:2 * ck + 2, :],
                                  in_=dram["pos_top"][ck])
            nc.sync.dma_start(out=augT[:, :, :], in_=dram["augs_top"][:, :])
            nc.sync.dma_start(out=pos["left"][:, :, :],
                              in_=dram["pos_left"][:, :])
            nc.sync.dma_start(out=augL[0:5, :, :], in_=dram["augs_left"][:, :])

            # PE warm-up (results never read): releases the HAM clock gate
            # during the DMA fill
            ps_w = ppool.tile([128, 512], mybir.dt.float32, tag="pw",
                              name="ps_warm", bufs=1)
            for _ in range(8):
                nc.tensor.matmul(ps_w[:, :], lhsT=scratch[:, 0:128],
                                 rhs=scratch[:, 128:640],
                                 start=True, stop=True)

            # cp (fp32 top-mask) and obf (bf16 min) tiles: one per row band
            cps = {}
            obfs = {}
            for mb in range(MB):
                wid = SEQ - ROW_LO[mb]
                cps[mb] = epool.tile([128, wid], mybir.dt.float32,
                                     tag=f"cp{mb}", name=f"cp{mb}")
                obfs[mb] = epool.tile([128, wid], mybir.dt.bfloat16,
                                      tag=f"ob{mb}", name=f"ob{mb}")

            def cp_slice(mb, c0, w):
                off = c0 - ROW_LO[mb]
                return cps[mb][:, off:off + w]

            def ob_slice(mb, c0, w):
                off = c0 - ROW_LO[mb]
                return obfs[mb][:, off:off + w]

            def psum_for(w, nm):
                # one shared full-bank tag; narrow blocks slice it
                t = ppool.tile([128, 512], mybir.dt.float32, tag="pmm",
                               name=nm, bufs=6)
                return t[:, :w]

            def dr_pass(ps, key, mb, c0, w, t0):
                m0 = mb * 128
                nc.tensor.matmul(
                    ps[:, :],
                    lhsT=pos[key][:, t0:t0 + 2, m0:m0 + 128],
                    rhs=pos[key][:, t0:t0 + 2, c0:c0 + w],
                    start=(t0 == 0), stop=False, perf_mode=DR,
                )

            def dr_passes(ps, key, mb, c0, w):
                for t0 in (0, 2):
                    dr_pass(ps, key, mb, c0, w, t0)

            def aug_pass(ps, aug_l, aug_r, mb, c0, w):
                m0 = mb * 128
                at = augT if aug_l == 0 else augL
                nc.tensor.matmul(
                    ps[:, :],
                    lhsT=at[:, 0:1, m0:m0 + 128],
                    rhs=at[:, 1:2, c0:c0 + w],
                    start=False, stop=True, skip_group_check=True,
                )

            # big blocks first, 256-wide blocks last (short MIN+store tail)
            ordA = [(mb, c0, w) for mb in range(MB)
                    for (c0, w) in ROW_BLOCKS[mb]]
            NB = len(ordA)

            def gemm_a(k):
                mb, c0, w = ordA[k]
                ps = psum_for(w, f"pt{mb}_{c0}")
                dr_passes(ps, "top", mb, c0, w)
                aug_pass(ps, 0, 1, mb, c0, w)
                nc.scalar.copy(cp_slice(mb, c0, w), ps[:, :])

            def gemm_b(k):
                mb, c0, w = ordA[k]
                ps = psum_for(w, f"pl{mb}_{c0}")
                dr_passes(ps, "left", mb, c0, w)
                aug_pass(ps, 2, 3, mb, c0, w)
                nc.vector.tensor_tensor(
                    out=ob_slice(mb, c0, w), in0=cp_slice(mb, c0, w),
                    in1=ps[:, :], op=mybir.AluOpType.min,
                )
                # one store per BAND (the ~650ns descriptor-gen per
                # dma_start serializes on the sequencer; 8 triggers beat 12)
                if c0 + w == SEQ:
                    r0 = mb * 128
                    lo = ROW_LO[mb]
                    eng = nc.sync if mb % 2 == 1 else nc.scalar
                    eng.dma_start(out=out[r0:r0 + 128, lo:SEQ],
                                  in_=obfs[mb][:, :])

            # top/left interleaved at block granularity: left GEMMs lag by
            # LAG blocks so the left-pos/aug DMA has landed, and the DVE MIN
            # chain spreads over the whole PE span instead of the back half
            LAG = 3
            # first 3 blocks: batch their kt-pair-0 DR passes so the PE can
            # start on ptop chunk 0 without waiting for chunk 1 / aug
            HEAD = 3
            head_ps = []
            for k in range(HEAD):
                mb, c0, w = ordA[k]
                ps = psum_for(w, f"pt{mb}_{c0}")
                head_ps.append(ps)
                dr_pass(ps, "top", mb, c0, w, 0)
            for k in range(HEAD):
                mb, c0, w = ordA[k]
                ps = head_ps[k]
                dr_pass(ps, "top", mb, c0, w, 2)
                aug_pass(ps, 0, 1, mb, c0, w)
                nc.scalar.copy(cp_slice(mb, c0, w), ps[:, :])
            for k in range(HEAD, LAG):
                gemm_a(k)
            for k in range(LAG, NB):
                gemm_b(k - LAG)
                gemm_a(k)
            for k in range(NB - LAG, NB):
                gemm_b(k)
    nc.compile()
    return nc


def _fp8_exact(x):
    f = x.astype(ml_dtypes.float8_e4m3).astype(np.float32)
    return np.array_equal(f, x)


def _aug_factor(ph):
    """Find c1*c2 == ph with c1, c2 fp8(e4m3)-exact; None if impossible."""
    for k in range(-6, 8):
        for m in range(8):
            c2 = np.float32(2.0 ** k) * np.float32(1 + m / 8.0)
            if c2 == 0:
                continue
            c1 = np.float32(ph) / c2
            cand = np.array([c1, c2], dtype=np.float32)
            if c1 * c2 == np.float32(ph) and _fp8_exact(cand):
                return float(c1), float(c2)
    return None


def _host_prep(zipped_top, zipped_left, indicator, p):
    """Build fp8 pos operands + aug k-tiles; returns (ins, fused)."""
    fp8 = ml_dtypes.float8_e4m3
    pos = {}
    s = {}
    for key, zipped in (("top", zipped_top), ("left", zipped_left)):
        b, seq, depth = zipped.shape
        oh = np.zeros((b, seq, TN + 1), dtype=np.float32)
        np.put_along_axis(oh, np.asarray(zipped, dtype=np.int64), 1.0, axis=2)
        oh = oh[..., :TN]
        s[key] = oh.sum(axis=2)                              # [b, seq]
        # [b, p, kt, j] k-tile-major
        pos[key] = oh.transpose(0, 2, 1).reshape(b, 4, 128, seq).transpose(
            0, 2, 1, 3)
    pad = (np.asarray(indicator) == 0).astype(np.float32)    # [b, seq]
    b, seq = pad.shape

    ph = np.float32(p) / np.float32(2.0)
    # s/2 is fp8-exact (multiples of 0.5 up to 2); need ph = c1*c2 exact
    fac = _aug_factor(ph) if _fp8_exact(s["top"] / 2) else None
    fused = fac is not None

    # aug k-tiles [b, 5, set, seq]; rows 5..127 are zeroed on-chip
    aug = np.zeros((b, 5, 4, seq), dtype=np.float32)
    for mi, key in enumerate(("top", "left")):
        sl, sr = 2 * mi, 2 * mi + 1
        if fused:
            c1, c2 = fac
            aug[:, 0, sl] = -s[key] / 2
            aug[:, 1, sl] = 1.0
            aug[:, 2, sl] = -c1 * pad
            aug[:, 3, sl] = -c1
            aug[:, 4, sl] = c1 * pad
            aug[:, 0, sr] = 1.0
            aug[:, 1, sr] = -s[key] / 2
            aug[:, 2, sr] = c2
            aug[:, 3, sr] = c2 * pad
            aug[:, 4, sr] = c2 * pad
        else:
            g = s[key] / np.float32(2.0) + ph * pad
            aug[:, 0, sl] = -g
            aug[:, 1, sl] = 1.0
            aug[:, 2, sl] = ph * pad
            aug[:, 0, sr] = 1.0
            aug[:, 1, sr] = -g
            aug[:, 2, sr] = pad
    aug_dt = fp8 if fused else ml_dtypes.bfloat16
    # chunk-major: [b, 2, 128, 2048] with chunk = mask (sets 01 | 23)
    augs5 = np.ascontiguousarray(
        aug.reshape(b, 5, 2, 2 * seq).transpose(0, 2, 1, 3))  # [b,2,5,2048]
    augs_top = np.zeros((b, 128, 2 * seq), dtype=np.float32)
    augs_top[:, 0:5] = augs5[:, 0]

    ins = {
        "pos_top": np.ascontiguousarray(
            pos["top"].reshape(b, 128, 2, 2 * seq).transpose(0, 2, 1, 3)
        ).astype(fp8),
        "pos_left": np.ascontiguousarray(
            pos["left"].reshape(b, 128, 4 * seq)).astype(fp8),
        "augs_top": augs_top.astype(aug_dt),
        "augs_left": np.ascontiguousarray(augs5[:, 1]).astype(aug_dt),
    }
    return ins, fused


def kernel(zipped_top, zipped_left, indicator, padding_dist):
    global LAST_RESULTS
    from concourse.bass_utils import run_bass_kernel_spmd

    p = float(np.asarray(padding_dist))
    ins, fused = _host_prep(
        np.asarray(zipped_top), np.asarray(zipped_left), indicator, p)

    if fused not in _NC_CACHE:
        _NC_CACHE[fused] = _build_nc(fused)
    nc = _NC_CACHE[fused]

    in_maps = [{k: v[c] for k, v in ins.items()} for c in range(N_CORES)]
    res = run_bass_kernel_spmd(
        nc, in_maps, core_ids=list(range(N_CORES)),
        trace=os.environ.get("BASS_TRACE", "") == "1",
    )
    LAST_RESULTS = res
    full = np.stack([res.results[c]["out"] for c in range(N_CORES)]).astype(
        np.float32
    )
    full *= np.float32(-2.0)
    # mirror the skipped below-diagonal region of each band
    for mb in range(MB):
        lo = ROW_LO[mb]
        if lo:
            r = slice(mb * 128, (mb + 1) * 128)
            full[:, r, :lo] = full[:, :lo, r].transpose(0, 2, 1)
    return full
